# revision 10
# baseline (speedup 1.0000x reference)
"""Trainium2 Bass kernel for nn_DGLRegressor (4-layer GCN + mean-pool + MLP head).

Math: each GraphConv layer is rewritten as
    h_{l+1} = relu( sum_e p_l'[src_e] + b_l*sqrtdeg_in[d] ),  p_l' = isr_out*.(h_l @ W_l),
with both degree normalizations folded into the table scaling (siso) and a
virtual-bias matmul (bbc x sdiag), so the one-hot edge matrix M is pure 0/1
for every layer; layer 4's isr_in[dst] factor is applied post-ReLU per tile.

Sharding (8 cores): nodes partitioned by dst range. Each core owns 98 tiles
of 128 dst nodes, processed in spans of 4 tiles. Per layer and span:
  - 4 merged dma_gathers (one per table chunk, ~2.5k rows each) fetch p rows
    by src; segments padded to full 128-row blocks (idx pad -> row 0, masked
    by the one-hot), so no memsets are needed,
  - one DVE is_equal builds the span's whole one-hot block run,
  - PE accumulates aggT += msg^T @ M per tile in PSUM (start: bias matmul),
  - ACT relu gives h^T; PE weight matmul + siso scale -> node-major p_{l+1},
    DMA'd out and AllGather'd piecewise (4 sub-gathers per layer).
Mean-pool: h4 * isr_in per tile (DVE), free-dim reduce + AllReduce; small
MLP head replicated on every core.
"""

import os
import numpy as np

import concourse.bacc as bacc
import concourse.bass as bass
import concourse.tile as tile
import concourse.mybir as mybir
import concourse.bass_utils as bass_utils

F16 = mybir.dt.float16
F32 = mybir.dt.float32
I16 = mybir.dt.int16

D = 128
N_CORES = 8

# Full-size problem config (hardcoded; kernel.py must be self-contained).
FULL = dict(
    n_real=100000,
    per_core_real=12500,
    sub_tiles=(27, 27, 24, 20),   # tiles per sub-slice; sum = 98 tiles/core
                                  # (small last slice: every layer's first
                                  # gathers wait on the LAST allgather piece)
    span=1,                       # tiles per gather span (1: single-packet SWDGE)
    r_cap=640,                    # max rows per (tile, chunk) segment
)

LAST_PERF = {}


# --------------------------------------------------------------------------
# host-side structure preprocessing (graph only: routing, degrees, layout)
# --------------------------------------------------------------------------

def _derive(cfg):
    sub_tiles = cfg["sub_tiles"]
    tiles = sum(sub_tiles)
    own = tiles * D
    cfg = dict(cfg)
    cfg["tiles"] = tiles
    cfg["own"] = own
    cfg["npad"] = own * N_CORES
    cfg["sub_base_t"] = tuple(np.cumsum((0,) + sub_tiles[:-1]).tolist())
    cfg["csize"] = tuple(st * D * N_CORES for st in sub_tiles)
    sub_of = np.zeros(tiles, np.int64)
    for s in range(4):
        sub_of[cfg["sub_base_t"][s]:cfg["sub_base_t"][s] + sub_tiles[s]] = s
    cfg["sub_of"] = sub_of
    G = cfg["span"]
    cfg["spans"] = [(t0, min(t0 + G, tiles)) for t0 in range(0, tiles, G)]
    return cfg


def _preprocess(cfg, x, src, dst):
    """Route edges, build per-core blocked metadata. Returns (meta, per-core arrays)."""
    n_real = cfg["n_real"]
    pcr = cfg["per_core_real"]
    tiles, own = cfg["tiles"], cfg["own"]
    sub_of, sub_base_t, sub_tiles = cfg["sub_of"], cfg["sub_base_t"], cfg["sub_tiles"]
    spans = cfg["spans"]

    src = np.asarray(src).astype(np.int64)
    dst = np.asarray(dst).astype(np.int64)
    deg_out = np.bincount(src, minlength=n_real).astype(np.float32)
    deg_in = np.bincount(dst, minlength=n_real).astype(np.float32)
    isr_out = 1.0 / np.sqrt(np.maximum(deg_out, 1.0))
    isr_in = 1.0 / np.sqrt(np.maximum(deg_in, 1.0))

    # node -> (core, tile, slot): contiguous core ranges; in-core snake deal by
    # in-degree to balance per-tile edge counts (tightens static gather sizes)
    pos = np.empty(n_real, np.int64)
    for c in range(N_CORES):
        nodes = np.arange(c * pcr, (c + 1) * pcr)
        order = np.argsort(-deg_in[nodes], kind="stable")
        rank = np.empty(pcr, np.int64)
        rank[order] = np.arange(pcr)
        rnd = rank // tiles
        idx = rank % tiles
        tl = np.where(rnd % 2 == 0, idx, tiles - 1 - idx)
        pos[nodes] = tl * D + rnd
    core_of = np.arange(n_real) // pcr
    tile_of = pos // D
    slot_of = pos % D
    # table row (global) and chunk-relative row for each real node
    s_of = sub_of[tile_of]
    u_of = tile_of - np.asarray(sub_base_t)[s_of]
    sub_rows = np.asarray([st * D for st in sub_tiles])
    rel_row = core_of * sub_rows[s_of] + u_of * D + slot_of
    assert rel_row.max() < 32768

    # per-(tile, chunk) counts across cores -> static R_tc, full-block padded
    e_core = dst // pcr
    e_tile = tile_of[dst]
    e_chunk = s_of[src]
    counts = np.zeros((N_CORES, tiles, 4), np.int64)
    np.add.at(counts, (e_core, e_tile, e_chunk), 1)
    maxc = counts.max(axis=0)                      # [tiles, 4]
    r_tc = ((maxc + 15) // 16 * 16).astype(np.int64)
    r_tc = np.maximum(r_tc, 16)
    assert r_tc.max() <= cfg["r_cap"], f"R_tc max {r_tc.max()} exceeds cap"
    b_tc = (r_tc + 127) // 128                     # blocks per (tile, chunk)
    s_tc = r_tc // 16                              # idx cols per (tile, chunk)

    # global stream order: span-major, chunk-major within span, tile within
    s_off = np.zeros((tiles, 4), np.int64)
    b_off = np.zeros((tiles, 4), np.int64)
    acc_s = acc_b = 0
    for (t0, t1) in spans:
        for c in range(4):
            for t in range(t0, t1):
                s_off[t, c] = acc_s
                b_off[t, c] = acc_b
                acc_s += s_tc[t, c]
                acc_b += b_tc[t, c]
    total_s, total_b = int(acc_s), int(acc_b)
    span_R = [[int(r_tc[t0:t1, c].sum()) for c in range(4)] for (t0, t1) in spans]
    span_B = [int(b_tc[t0:t1, :].sum()) for (t0, t1) in spans]
    bspan_max = int(max(span_B))

    # per-core metadata arrays
    per_core = []
    for c in range(N_CORES):
        m = e_core == c
        es, ed = src[m], dst[m]
        et, ech = e_tile[m], e_chunk[m]
        key = et * 4 + ech
        # within each (tile, chunk) group, sort edges by table row for
        # HBM locality of the gathers
        order = np.lexsort((rel_row[es], key))
        es, ed, et, ech = es[order], ed[order], et[order], ech[order]
        key = key[order]
        grp_start = np.searchsorted(key, np.arange(tiles * 4))
        rank = np.arange(len(key)) - grp_start[key]
        boff_e = b_off[et, ech]
        soff_e = s_off[et, ech]

        idx16 = np.zeros((16, total_s), np.int16)
        dstl = np.full((128, total_b), 200.0, np.float16)  # 200 never matches iota
        # idx: logical rank i -> [i%16, soff + i//16]; pads stay 0 (row 0,
        # masked by dstl=200 in the one-hot)
        idx16[rank % 16, soff_e + rank // 16] = rel_row[es].astype(np.int16)
        # dst slot: rank i -> partition i%128, block boff + i//128
        dstl[rank % 128, boff_e + rank // 128] = slot_of[ed].astype(np.float16)
        idx_full = np.tile(idx16, (8, 1))
        per_core.append(dict(idx=idx_full, dstl=dstl))

    meta = dict(r_tc=r_tc, b_tc=b_tc, s_tc=s_tc, s_off=s_off, b_off=b_off,
                total_s=total_s, total_b=total_b, span_R=span_R,
                bspan_max=bspan_max, n_pad_nodes=N_CORES * own - n_real)

    # x_own_T fp16 [128, own] per core (feature-major), pads zero
    col = tile_of * D + slot_of
    xt = np.asarray(x, dtype=np.float32)
    sqrt_in = np.sqrt(np.maximum(deg_in, 1.0)).astype(np.float32)
    for c in range(N_CORES):
        xo = np.zeros((D, own), np.float16)
        nodes = np.arange(c * pcr, (c + 1) * pcr)
        xo[:, col[nodes]] = xt[nodes].T.astype(np.float16)
        per_core[c]["xT"] = xo
        isro = np.ones((D, tiles), np.float32)
        siso = np.ones((D, tiles), np.float32)
        isro[slot_of[nodes], tile_of[nodes]] = isr_out[nodes]
        siso[slot_of[nodes], tile_of[nodes]] = (isr_out[nodes] * isr_in[nodes])
        per_core[c]["isro"] = isro
        per_core[c]["siso"] = siso
        sd = np.zeros((D, tiles * D), np.float16)
        sd[slot_of[nodes], tile_of[nodes] * D + slot_of[nodes]] = sqrt_in[nodes].astype(np.float16)
        # pad slots: sqrtdeg = 1 on the diagonal (bias still applies to pad nodes)
        padmask = np.ones((D, tiles), bool)
        padmask[slot_of[nodes], tile_of[nodes]] = False
        ps, pt = np.nonzero(padmask)
        sd[ps, pt * D + ps] = 1.0
        per_core[c]["sdiag"] = sd
        # isr_in per dst column, replicated across partitions (layer-4 scale)
        ir = np.ones((D, tiles * D), np.float16)
        ir[:, col[nodes]] = isr_in[nodes].astype(np.float16)[None, :]
        per_core[c]["isrin"] = ir

    # iota constant [128, bspan_max*128] fp16: value = position within block
    iota = np.tile(np.arange(D, dtype=np.float16)[None, :], (D, bspan_max))
    meta["iota"] = iota
    return meta, per_core


# --------------------------------------------------------------------------
# device program
# --------------------------------------------------------------------------

def _build(cfg, meta):
    tiles = cfg["tiles"]
    sub_tiles, sub_base_t = cfg["sub_tiles"], cfg["sub_base_t"]
    sub_of = cfg["sub_of"]
    csize = cfg["csize"]
    spans = cfg["spans"]
    b_tc = meta["b_tc"]
    s_off, b_off = meta["s_off"], meta["b_off"]
    total_s, total_b = meta["total_s"], meta["total_b"]
    span_R = meta["span_R"]
    bspan_max = meta["bspan_max"]
    n_pad = meta["n_pad_nodes"]
    n_real = cfg["n_real"]
    own = cfg["own"]

    nc = bacc.Bacc("TRN2", target_bir_lowering=False, debug=False,
                   num_devices=N_CORES, num_swdge_queues=4)

    # inputs
    xT = nc.dram_tensor("xT", [D, own], F16, kind="ExternalInput").ap()
    idx_t = nc.dram_tensor("idx", [128, total_s], I16, kind="ExternalInput").ap()
    dstl_t = nc.dram_tensor("dstl", [128, total_b], F16, kind="ExternalInput").ap()
    iota_t = nc.dram_tensor("iota", [128, bspan_max * D], F16, kind="ExternalInput").ap()
    isro_t = nc.dram_tensor("isro", [D, tiles], F32, kind="ExternalInput").ap()
    siso_t = nc.dram_tensor("siso", [D, tiles], F32, kind="ExternalInput").ap()
    sdiag_t = nc.dram_tensor("sdiag", [D, tiles * D], F16, kind="ExternalInput").ap()
    isrin_t = nc.dram_tensor("isrin", [D, tiles * D], F16, kind="ExternalInput").ap()
    bbc_t = nc.dram_tensor("bbc", [D, 4 * D], F16, kind="ExternalInput").ap()
    W16 = [nc.dram_tensor(f"W{i+1}", [D, D], F16, kind="ExternalInput").ap() for i in range(4)]
    Bv = [nc.dram_tensor(f"b{i+1}", [D, 1], F32, kind="ExternalInput").ap() for i in range(4)]
    Wl1 = nc.dram_tensor("Wl1", [D, D], F32, kind="ExternalInput").ap()
    Wl2 = nc.dram_tensor("Wl2", [D, D], F32, kind="ExternalInput").ap()
    Wo = nc.dram_tensor("Wo", [D, 1], F32, kind="ExternalInput").ap()
    bl1 = nc.dram_tensor("bl1", [D, 1], F32, kind="ExternalInput").ap()
    bl2 = nc.dram_tensor("bl2", [D, 1], F32, kind="ExternalInput").ap()
    bo = nc.dram_tensor("bo", [D, 1], F32, kind="ExternalInput").ap()
    out_t = nc.dram_tensor("out", [D, 1], F32, kind="ExternalOutput").ap()

    # internal DRAM: per layer, 4 own pieces + 4 gathered pieces
    pown = [[nc.dram_tensor(f"pown{l}_{s}", [sub_tiles[s] * D, D], F16)
             for s in range(4)] for l in range(4)]
    pfull = [[nc.dram_tensor(f"pfull{l}_{s}", [csize[s], D], F16)
              for s in range(4)] for l in range(4)]
    pool_b = [nc.dram_tensor("pool_in", [D, 1], F32), nc.dram_tensor("pool_out", [D, 1], F32)]

    RG = [list(range(N_CORES))]
    qctr = [0]
    G = cfg["span"]

    with tile.TileContext(nc) as tc:
        with tc.tile_pool(name="const", bufs=1) as constp, \
             tc.tile_pool(name="xt", bufs=3) as xtp, \
             tc.tile_pool(name="sd", bufs=3) as sdp, \
             tc.tile_pool(name="msg", bufs=10) as msgp, \
             tc.tile_pool(name="eqm", bufs=3) as eqp, \
             tc.tile_pool(name="psA", bufs=2, space="PSUM") as psA, \
             tc.tile_pool(name="psB", bufs=2, space="PSUM") as psB, \
             tc.tile_pool(name="hpo", bufs=6) as hp:

            # resident constants / metadata
            idx_sb = constp.tile([128, total_s], I16)
            nc.sync.dma_start(out=idx_sb[:], in_=idx_t[:])
            dstl_sb = constp.tile([128, total_b], F16)
            nc.sync.dma_start(out=dstl_sb[:], in_=dstl_t[:])
            iota_sb = constp.tile([128, bspan_max, D], F16)
            nc.sync.dma_start(out=iota_sb[:, :, :], in_=iota_t[:])
            isro_sb = constp.tile([D, tiles], F32)
            nc.sync.dma_start(out=isro_sb[:], in_=isro_t[:])
            siso_sb = constp.tile([D, tiles], F32)
            nc.sync.dma_start(out=siso_sb[:], in_=siso_t[:])
            bbc_sb = constp.tile([D, 4 * D], F16)
            nc.sync.dma_start(out=bbc_sb[:], in_=bbc_t[:])
            W_sb = []
            for i in range(4):
                w = constp.tile([D, D], F16, tag=f"W{i}")
                nc.sync.dma_start(out=w[:], in_=W16[i][:])
                W_sb.append(w)
            b_sb = []
            for i in range(4):
                b = constp.tile([D, 1], F32, tag=f"b{i}")
                nc.sync.dma_start(out=b[:], in_=Bv[i][:])
                b_sb.append(b)
            Wl1_sb = constp.tile([D, D], F32); nc.sync.dma_start(out=Wl1_sb[:], in_=Wl1[:])
            Wl2_sb = constp.tile([D, D], F32); nc.sync.dma_start(out=Wl2_sb[:], in_=Wl2[:])
            Wo_sb = constp.tile([D, 1], F32); nc.sync.dma_start(out=Wo_sb[:], in_=Wo[:])
            bl1_sb = constp.tile([D, 1], F32); nc.sync.dma_start(out=bl1_sb[:], in_=bl1[:])
            bl2_sb = constp.tile([D, 1], F32); nc.sync.dma_start(out=bl2_sb[:], in_=bl2[:])
            bo_sb = constp.tile([D, 1], F32); nc.sync.dma_start(out=bo_sb[:], in_=bo[:])
            pool_parts = constp.tile([D, tiles], F32)

            # one-time init of the rotating msg buffers: gather tails beyond
            # each segment's R rows stay stale-but-finite afterwards (masked
            # by the one-hot), but first use must not see NaN bit patterns
            for _ in range(10):
                m0 = msgp.tile([128, bspan_max, D], F16, tag="msg")
                nc.vector.memset(m0[:, :, :], 0)

            # ---- stage A: p1 = x @ W1 per own tile, piecewise allgather ----
            for t in range(tiles):
                s = int(sub_of[t]); u = t - sub_base_t[s]
                xt_sb = xtp.tile([D, D], F16, tag="xt")
                nc.sync.dma_start(out=xt_sb[:], in_=xT[:, t * D:(t + 1) * D])
                pp = psB.tile([D, D], F32, tag="pps")
                nc.tensor.matmul(out=pp[:], lhsT=xt_sb[:],
                                 rhs=W_sb[0][:], start=True, stop=True)
                po = hp.tile([D, D], F16, tag="po")
                nc.scalar.activation(po[:], pp[:], mybir.ActivationFunctionType.Copy,
                                     scale=isro_sb[:, t:t + 1])
                nc.sync.dma_start(out=pown[0][s][u * D:(u + 1) * D, :], in_=po[:])
                if u == sub_tiles[s] - 1:
                    nc.gpsimd.collective_compute(
                        "AllGather", mybir.AluOpType.bypass, replica_groups=RG,
                        ins=[pown[0][s].ap().opt()], outs=[pfull[0][s].ap().opt()])

            # ---- stage B: 4 conv layers, span-merged gathers ----
            for l in range(4):
                for sp, (t0, t1) in enumerate(spans):
                    b0 = int(b_off[t0, 0])
                    Bsp = sum(int(b_tc[t, c]) for t in range(t0, t1) for c in range(4))
                    # per-span rhs metadata (sqrtdeg diag; layer 4: isr_in row)
                    sd_sp = sdp.tile([D, G, D], F16, tag="sd")
                    nc.sync.dma_start(out=sd_sp[:, :t1 - t0, :],
                                      in_=sdiag_t[:, t0 * D:t1 * D])
                    if l == 3:
                        ir_sp = sdp.tile([D, G, D], F16, tag="ir")
                        nc.sync.dma_start(out=ir_sp[:, :t1 - t0, :],
                                          in_=isrin_t[:, t0 * D:t1 * D])
                    msg = msgp.tile([128, bspan_max, D], F16, tag="msg")
                    for c in range(4):
                        R = span_R[sp][c]
                        Bc = (R + 127) // 128
                        bo_c = int(b_off[t0, c]) - b0
                        so_c = int(s_off[t0, c])
                        nc.gpsimd.dma_gather(
                            out_ap=msg[:, bo_c:bo_c + Bc, :],
                            in_ap=pfull[l][c].ap()[:, :],
                            idxs_ap=idx_sb[:, so_c:so_c + R // 16],
                            num_idxs=R, num_idxs_reg=R, elem_size=D,
                            queue_num=qctr[0] % 4)
                        qctr[0] += 1
                    eq = eqp.tile([128, bspan_max, D], F16, tag="eq")
                    nc.vector.tensor_tensor(
                        out=eq[:, :Bsp, :],
                        in0=dstl_sb[:, b0:b0 + Bsp, None].to_broadcast([128, Bsp, D]),
                        in1=iota_sb[:, :Bsp, :],
                        op=mybir.AluOpType.is_equal)
                    for t in range(t0, t1):
                        s = int(sub_of[t]); u = t - sub_base_t[s]
                        agg = psA.tile([D, D], F32, tag="agg")
                        # virtual bias block: aggT += b_{l+1}[f] * sqrtdeg_in[d]
                        nc.tensor.matmul(out=agg[:], lhsT=bbc_sb[:, l * D:(l + 1) * D],
                                         rhs=sd_sp[:, t - t0, :],
                                         start=True, stop=False)
                        for c in range(4):
                            nblk = int(b_tc[t, c])
                            bl_c = int(b_off[t, c]) - b0
                            for b in range(nblk):
                                nc.tensor.matmul(
                                    out=agg[:], lhsT=msg[:, bl_c + b, :],
                                    rhs=eq[:, bl_c + b, :],
                                    start=False, stop=(c == 3 and b == nblk - 1))
                        h = hp.tile([D, D], F16, tag="h")
                        nc.scalar.activation(h[:], agg[:], mybir.ActivationFunctionType.Relu)
                        if l < 3:
                            pp = psB.tile([D, D], F32, tag="pps")
                            nc.tensor.matmul(out=pp[:], lhsT=h[:], rhs=W_sb[l + 1][:],
                                             start=True, stop=True)
                            po = hp.tile([D, D], F16, tag="po")
                            nc.scalar.activation(po[:], pp[:], mybir.ActivationFunctionType.Copy,
                                                 scale=siso_sb[:, t:t + 1])
                            nc.sync.dma_start(out=pown[l + 1][s][u * D:(u + 1) * D, :], in_=po[:])
                            if u == sub_tiles[s] - 1:
                                nc.gpsimd.collective_compute(
                                    "AllGather", mybir.AluOpType.bypass, replica_groups=RG,
                                    ins=[pown[l + 1][s].ap().opt()],
                                    outs=[pfull[l + 1][s].ap().opt()])
                        else:
                            hw = hp.tile([D, D], F16, tag="hw")
                            nc.vector.tensor_tensor(out=hw[:], in0=h[:],
                                                    in1=ir_sp[:, t - t0, :],
                                                    op=mybir.AluOpType.mult)
                            nc.vector.tensor_reduce(out=pool_parts[:, t:t + 1], in_=hw[:],
                                                    axis=mybir.AxisListType.X,
                                                    op=mybir.AluOpType.add)

            # ---- pooling + head (replicated on every core) ----
            psum_pool = constp.tile([D, 1], F32)
            nc.vector.tensor_reduce(out=psum_pool[:], in_=pool_parts[:],
                                    axis=mybir.AxisListType.X, op=mybir.AluOpType.add)
            nc.sync.dma_start(out=pool_b[0].ap()[:, :], in_=psum_pool[:])
            nc.gpsimd.collective_compute(
                "AllReduce", mybir.AluOpType.add, replica_groups=RG,
                ins=[pool_b[0].ap().opt()], outs=[pool_b[1].ap().opt()])
            sum_all = constp.tile([D, 1], F32)
            nc.sync.dma_start(out=sum_all[:], in_=pool_b[1].ap()[:, :])
            # hg = (sum_all - n_pad*relu(b4)) / n_real
            relu_b4 = constp.tile([D, 1], F32)
            nc.scalar.activation(relu_b4[:], b_sb[3][:], mybir.ActivationFunctionType.Relu)
            corr = constp.tile([D, 1], F32)
            nc.vector.tensor_scalar_mul(out=corr[:], in0=relu_b4[:], scalar1=-float(n_pad))
            hg = constp.tile([D, 1], F32)
            nc.vector.tensor_tensor(out=hg[:], in0=sum_all[:], in1=corr[:],
                                    op=mybir.AluOpType.add)
            nc.vector.tensor_scalar_mul(out=hg[:], in0=hg[:], scalar1=1.0 / n_real)

            ps1 = psA.tile([D, 1], F32, tag="head")
            nc.tensor.matmul(out=ps1[:], lhsT=Wl1_sb[:], rhs=hg[:], start=True, stop=True)
            hg1 = constp.tile([D, 1], F32)
            nc.scalar.activation(hg1[:], ps1[:], mybir.ActivationFunctionType.Relu,
                                 bias=bl1_sb[:], scale=1.0)
            ps2 = psA.tile([D, 1], F32, tag="head")
            nc.tensor.matmul(out=ps2[:], lhsT=Wl2_sb[:], rhs=hg1[:], start=True, stop=True)
            hg2 = constp.tile([D, 1], F32)
            nc.scalar.activation(hg2[:], ps2[:], mybir.ActivationFunctionType.Relu,
                                 bias=bl2_sb[:], scale=1.0)
            ps3 = psA.tile([1, 1], F32, tag="head1")
            nc.tensor.matmul(out=ps3[:], lhsT=Wo_sb[:], rhs=hg2[:], start=True, stop=True)
            res = constp.tile([1, 1], F32)
            nc.vector.tensor_tensor(out=res[:], in0=ps3[:], in1=bo_sb[0:1, :],
                                    op=mybir.AluOpType.add)
            nc.sync.dma_start(out=out_t[0:1, :], in_=res[:])

    nc.compile()
    return nc


# --------------------------------------------------------------------------
# entry point
# --------------------------------------------------------------------------

def _run(cfg, x, src, dst, W1, b1, W2, b2, W3, b3, W4, b4, Wl1, bl1, Wl2, bl2, Wo, bo,
         check_sim=False):
    cfg = _derive(cfg)
    meta, per_core = _preprocess(cfg, x, src, dst)
    nc = _build(cfg, meta)

    Ws = [W1, W2, W3, W4]
    bs = [b1, b2, b3, b4]
    common = {}
    for i in range(4):
        common[f"W{i+1}"] = np.asarray(Ws[i], np.float32).astype(np.float16)
        common[f"b{i+1}"] = np.asarray(bs[i], np.float32).reshape(D, 1)
    common["Wl1"] = np.asarray(Wl1, np.float32)
    common["Wl2"] = np.asarray(Wl2, np.float32)
    common["Wo"] = np.asarray(Wo, np.float32).reshape(D, 1)
    common["bl1"] = np.asarray(bl1, np.float32).reshape(D, 1)
    common["bl2"] = np.asarray(bl2, np.float32).reshape(D, 1)
    common["bo"] = np.tile(np.asarray(bo, np.float32).reshape(1, 1), (D, 1))
    common["iota"] = meta["iota"]
    common["bbc"] = np.concatenate(
        [np.tile(np.asarray(bs[i], np.float32).astype(np.float16)[None, :], (D, 1))
         for i in range(4)], axis=1)

    in_maps = []
    for c in range(N_CORES):
        m = dict(common)
        m["xT"] = per_core[c]["xT"]
        m["idx"] = per_core[c]["idx"]
        m["dstl"] = per_core[c]["dstl"]
        m["isro"] = per_core[c]["isro"]
        m["siso"] = per_core[c]["siso"]
        m["sdiag"] = per_core[c]["sdiag"]
        m["isrin"] = per_core[c]["isrin"]
        in_maps.append(m)

    res = bass_utils.run_bass_kernel_spmd(
        nc, in_maps, core_ids=list(range(N_CORES)),
        trace=bool(int(os.environ.get("GCN_TRACE", "1"))))
    LAST_PERF.clear()
    LAST_PERF["exec_time_ns"] = res.exec_time_ns
    LAST_PERF["trace"] = res.instructions_and_trace[1] if res.instructions_and_trace else None
    return res.results[0]["out"][0:1, :].astype(np.float32)


def kernel(x, src, dst, W1, b1, W2, b2, W3, b3, W4, b4, Wl1, bl1, Wl2, bl2, Wo, bo):
    return _run(FULL, x, src, dst, W1, b1, W2, b2, W3, b3, W4, b4,
                Wl1, bl1, Wl2, bl2, Wo, bo)
